# revision 52
# baseline (speedup 1.0000x reference)
"""KANConv2d Trainium2 kernel (8-core data-parallel over batch).

Math: the efficient-kan B-spline bases on the uniform grid (knots at
u = 0..11, u = x/0.4 + 5.5) are combinations of truncated cubic powers.
The naive one-sided stack relu(u-m)^3 cancels ~2200-magnitude terms
down to O(1), which float32r's ~11-bit mantissa cannot survive (10%
error). Instead bases g=0..3 use right-side CLAMPED powers
relu(m - max(u,0))^3 and bases g=4..7 left-side relu(min(u,11) - m)^3:
the clamps make both telescopes exactly zero outside support and cap
term magnitudes at 343, so fp32r matmuls (4x the fp32 PE rate) keep
the end-to-end error at ~1.5e-2. The layer becomes a 3x3 conv over
15*64 feature channels [silu(x); 14 clamped cubes] with host-folded
weights, run as shifted fp32r matmuls accumulating in PSUM over a
58-wide zero-padded flattened feature map.

Feature tiles (128 partitions = 2 channel groups of 64):
  t0    = [silu(map); silu(map shifted left 1 col)]
  t1..7 = clamped-cube pairs (relu via per-partition bias against the
          shared precursors v0 = relu(u), w11 = relu(11-u))
The shifted silu copy lets one 128-row matmul cover taps kw=0 and kw=1
simultaneously, so t0 needs 6 matmul jobs per 9 taps instead of 9+
half-empty ones.  Matmul rhs uses a strided [8 rows x 56 cols] view so
no garbage pad columns are streamed (448 rows/job/bank). PE warm-up
matmuls keep the busy-streak alive so real matmuls are cost-priced at
full p-state (dispatch-time pricing against the 36-deep PE queue).
"""
import math
import numpy as np

import concourse.bass as bass
import concourse.mybir as mybir
import concourse.tile as tile
from concourse.tile import TileContext

# ---- problem constants (hardcoded per harness contract) ----
B, C, H, W = 8, 64, 56, 56
OC = 128
GRID_SIZE, SPLINE_ORDER = 5, 3
GRID_LO, GRID_HI = -1.0, 1.0
HSTEP = (GRID_HI - GRID_LO) / GRID_SIZE        # 0.4
T0 = GRID_LO - SPLINE_ORDER * HSTEP            # -2.2
NM = GRID_SIZE + 2 * SPLINE_ORDER + 1          # 12 truncated powers
WP = W + 2                                     # 58 padded width
PADFLAT = WP * WP + 4                          # 3368 padded/rounded
NROW = 8                                       # out rows per psum bank
NCHUNK = NROW * W                              # 448 (<=512 psum fp32)
NCH = 7                                        # chunks: 7*8 = 56 rows
NT = 8                                         # feature tiles
F32 = mybir.dt.float32
F32R = mybir.dt.float32r
M_DT = mybir.dt.float32r   # matmul operand dtype (float32r or float32)
F32R_TILES = set(range(8))  # feature tiles using f32r (debug bisect)


NWARM_TINY = 30                                # burn the 36-deep price burst
NWARM = 30                                     # PE warm-up matmuls (medium)
NWARM2 = 0                                     # pad before t0half banks 3-6
NWARM3 = 7                                     # pad before t0full
NWARM4 = 28                                    # pad before t1 (cube pipeline fill)
CA = 1682                                      # column split (banks 0-2 | 3-6)

# Two-sided clamped truncated-power features (f32r-safe): bases g=0..3 via
# right-side powers relu(m - max(u,0))^3 (m=1..7), bases g=4..7 via
# left-side powers relu(min(u,11) - m)^3 (m=4..10). Both telescopes are
# exactly zero outside support thanks to the clamps, and term magnitudes
# stay <= 343 so float32r's ~11-bit mantissa noise stays ~1e-2 under the
# massive-cancellation blowup of the naive one-sided stack.
# feature list: ("R", m) -> relu(m - v0)^3 with v0 = relu(u)
#               ("L", m) -> relu((11-m) - w11)^3 with w11 = relu(11-u)
_FEATS = [("R", m) for m in range(1, 8)] + [("L", m) for m in range(4, 11)]
# cube tile pairing: (lo feature idx, hi feature idx) per tile 1..7
_PAIRS = [(0, 1), (2, 3), (4, 5), (6, 7), (8, 9), (10, 11), (12, 13)]


def _jobs():
    """Matmul job list: (tile, rows, off, blocks). blocks[i] covers lhsT
    rows 64i:64i+64; ("base", kh, kw) or ("feat", fidx, kh, kw)."""
    jobs = []
    # t0 half jobs first (need only the unshifted silu half)
    for kh in range(3):
        jobs.append((0, 64, kh * WP + 2, [("base", kh, 2)]))
    # t0 full jobs: rows 0:64 tap kw=0, rows 64:128 (shifted copy) tap kw=1
    for kh in range(3):
        jobs.append((0, 128, kh * WP, [("base", kh, 0), ("base", kh, 1)]))
    for i in range(1, NT):
        lo, hi = _PAIRS[i - 1]
        for s in range(9):
            kh, kw = s // 3, s % 3
            jobs.append((i, 128, kh * WP + kw,
                         [("feat", lo, kh, kw), ("feat", hi, kh, kw)]))
    return jobs


_JOBS = _jobs()
NJ = len(_JOBS)                                # 60
# jobs per feature tile (for weight DMA splitting)
_TILE_J0 = [min(j for j, job in enumerate(_JOBS) if job[0] == t) for t in range(NT)]
_TILE_J1 = [max(j for j, job in enumerate(_JOBS) if job[0] == t) + 1 for t in range(NT)]


def _patch_tile_drain():
    """walrus in this container rejects sem waits on InstDrain (CTRL_NO
    struct): move the end-of-kernel drain waits onto single-wait NOPs."""
    import bass_rust

    def _drain_and_barrier(self, tick_clock, wait_clock):
        collector = self.nc.sync.nop(nofuse=True, hint="drain_waits")
        wait_clock.add_sem_waits(
            collector.ins, bass_rust.ScopedClock({None: tick_clock.global_clock})
        )
        waits = list(collector.ins.sync_info.on_wait)
        collector.ins.sync_info = mybir.SyncInfo(on_wait=waits[:1], on_update=[])
        for w in waits[1:]:
            n = self.nc.sync.nop(nofuse=True, hint="drain_waits")
            n.ins.sync_info = mybir.SyncInfo(on_wait=[w], on_update=[])
        self.nc.sync.drain()
        self.nc.all_engine_barrier()
        popped = self.nc._tile_sem_poison_stack.pop()
        assert popped is self._sem_poison
        self.nc.clear_and_free_semaphores(list(self.sems.allocated().values()))
        self.nc.all_engine_barrier()

    TileContext._drain_and_barrier = _drain_and_barrier


_patch_tile_drain()


def _split_excess_waits(nc):
    """This walrus caps sync waits at 1/instruction (2 for EventSemaphore).
    Spill excess waits onto EventSemaphore insts inserted just before the
    overloaded instruction on the same engine."""
    import bass_rust

    counter = [0]
    for func in nc.m.functions:
        for bb in func.blocks:
            insts = bb.instructions
            out = []
            changed = False
            for inst in insts:
                si = getattr(inst, "sync_info", None)
                waits = list(si.on_wait) if si is not None else []
                cap = 2 if isinstance(inst, bass_rust.InstEventSemaphore) else 1
                if len(waits) > cap:
                    excess = waits[cap:]
                    for i in range(0, len(excess), 2):
                        counter[0] += 1
                        ev = bass_rust.InstEventSemaphore(
                            name=f"evspill-{counter[0]}",
                            engine=inst.engine,
                            ins=[], outs=[],
                            sync_info=mybir.SyncInfo(
                                on_wait=excess[i:i + 2], on_update=[]),
                        )
                        out.append(ev)
                    inst.sync_info = mybir.SyncInfo(
                        on_wait=waits[:cap], on_update=list(si.on_update))
                    changed = True
                out.append(inst)
            if changed:
                bb.instructions = out


def _feat_bias(fidx):
    """relu argument: bias - precursor (precursor = v0 for R, w11 for L)."""
    side, m = _FEATS[fidx]
    return float(m) if side == "R" else float(11 - m)


def _host_weights(base_weight, spline_weight, spline_scaler):
    """Fold spline bases into per-job lhsT blocks Wt[row, j, o]."""
    NB = GRID_SIZE + SPLINE_ORDER
    Tnew = np.zeros((NB, len(_FEATS)), dtype=np.float64)
    fmap = {f: i for i, f in enumerate(_FEATS)}
    for g in range(NB):
        for r in range(SPLINE_ORDER + 2):
            cr = ((-1) ** r) * math.comb(SPLINE_ORDER + 1, r) / 6.0
            key = ("R", g + 4 - r) if g <= 3 else ("L", g + r)
            if key in fmap:
                Tnew[g, fmap[key]] += cr
            # dropped features (R0, L11) are identically zero
    scaled = spline_weight.astype(np.float64) * spline_scaler.astype(np.float64)[..., None]
    W2 = np.einsum("oig,gf->oif", scaled, Tnew)                 # (O, 576, 14)
    W24 = W2.reshape(OC, C, 3, 3, len(_FEATS))
    bw4 = base_weight.astype(np.float64).reshape(OC, C, 3, 3)
    wt = np.zeros((128, NJ, OC), dtype=np.float32)
    for j, (t, rows, off, blocks) in enumerate(_JOBS):
        for bi, blk in enumerate(blocks):
            if blk[0] == "base":
                _, kh, kw = blk
                blockw = bw4[:, :, kh, kw]                      # (O, C)
            else:
                _, fidx, kh, kw = blk
                blockw = W24[:, :, kh, kw, fidx]                # (O, C)
            wt[64 * bi:64 * bi + 64, j, :] = blockw.T.astype(np.float32)
    bias = np.zeros((128, NT - 1), dtype=np.float32)
    for i in range(1, NT):
        lo, hi = _PAIRS[i - 1]
        bias[0:64, i - 1] = _feat_bias(lo)
        bias[64:128, i - 1] = _feat_bias(hi)
    return wt, bias


def _build_nc():
    nc = bass.Bass()
    tdt = [M_DT if t in F32R_TILES else F32 for t in range(NT)]
    x_in = nc.declare_dram_parameter("x", [C, H, W], F32, isOutput=False)
    wt_in = nc.declare_dram_parameter("wt", [128, NJ, OC], F32, isOutput=False)
    out = nc.declare_dram_parameter("out", [OC, H, W], F32, isOutput=True)

    AF = mybir.ActivationFunctionType
    with TileContext(nc) as tc:
        with (
            tc.tile_pool(name="w", bufs=1) as wpool,
            tc.tile_pool(name="xf", bufs=1) as xfpool,
            tc.tile_pool(name="ftc", bufs=4) as ftcpool,
            tc.tile_pool(name="tmp", bufs=2) as tmppool,
            tc.tile_pool(name="ob", bufs=1) as opool,
            tc.tile_pool(name="psum", bufs=1, space="PSUM") as psumpool,
        ):
            # relu bias vector built by on-chip memsets (no DMA slot)
            bias_sb = wpool.tile([128, NT], F32, tag="bias_sb")
            for i in range(1, NT):
                lo, hi = _PAIRS[i - 1]
                nc.gpsimd.memset(bias_sb[0:64, i - 1:i], _feat_bias(lo))
                nc.gpsimd.memset(bias_sb[64:128, i - 1:i], _feat_bias(hi))
            nc.gpsimd.memset(bias_sb[:, NT - 1:NT], -T0 / HSTEP)  # precursor bias 5.5
            xpad = xfpool.tile([128, PADFLAT], F32, tag="xpad")
            xv = xpad[:, :WP * WP].rearrange("p (r c) -> p r c", c=WP)
            # zero only the pad border of rows 0:64 (the interior is about
            # to be DMA'd over; a full memset would delay that DMA). Rows
            # 64:128 get everything via the on-chip duplication DMA.
            nc.gpsimd.memset(xpad[0:C, 0:WP - 1], 0.0)
            # right border of row r + left border of row r+1 are adjacent
            nc.gpsimd.memset(
                xpad[0:C, WP - 1:WP - 1 + 57 * WP]
                .rearrange("p (r c) -> p r c", c=WP)[:, :, 0:2], 0.0)
            nc.gpsimd.memset(xpad[0:C, 57 * WP + 1:PADFLAT], 0.0)

            # PE warm-up: instruction costs are priced at engine dispatch
            # against the PE busy-ramp (pe_busy_start = start of the current
            # busy streak), and the 36-deep PE queues are priced in a burst
            # at streak start. Any PE idle gap therefore poisons the next
            # ~36 matmuls with the slow-p-state price. The warm block keeps
            # PE busy from ~2us until the first real matmul is ready.
            warm = wpool.tile([64, 128], F32, tag="warm")
            nc.gpsimd.memset(warm[:], 0.0)
            psum_w = psumpool.tile([64, 64], F32, tag="pwarm")

            def warms(n, ap=64):
                for _ in range(n):
                    nc.tensor.matmul(psum_w[0:64, 0:ap], warm[0:64, 0:64].bitcast(M_DT),
                                     warm[0:64, 64:64 + ap].bitcast(M_DT),
                                     start=True, stop=True)

            warms(NWARM_TINY, ap=8)
            warms(NWARM)

            w_sb = [wpool.tile([128, (_TILE_J1[t] - _TILE_J0[t]) * OC], tdt[t],
                               tag=f"w{t}", name=f"w{t}") for t in range(NT)]

            # Single SP queue; device transfer order == this order (the dup
            # DMAs are "ready" by the time the static scheduler reaches
            # them, so nothing gets hoisted past them). The t0-half-job
            # weights (64 rows, 3 jobs) ride a tiny early DMA so the first
            # real matmul gate is x + silu, not the full wt0 transfer.
            nc.sync.dma_start(xv[0:C, 1:29, 1:W + 1], x_in[:, 0:28, :])
            nc.sync.dma_start(w_sb[0][0:64, 0:3 * OC],
                              wt_in[0:64, 0:3, :].rearrange("p j o -> p (j o)").bitcast(tdt[0]))
            nc.sync.dma_start(xpad[64:128, 0:CA], xpad[0:C, 0:CA])
            nc.sync.dma_start(xv[0:C, 29:H + 1, 1:W + 1], x_in[:, 28:H, :])

            ft = [xfpool.tile([128, PADFLAT], tdt[0], tag="ft0", name="ft0")]
            ft += [ftcpool.tile([128, PADFLAT], tdt[t], tag="ftc", name=f"ft{t}")
                   for t in range(1, NT)]
            # Early passes split at the bank-2/3 column boundary so matmuls
            # on banks 0-2 can start while the B half is still computing.
            nc.scalar.activation(ft[0][0:64, 0:CA], xpad[0:64, 0:CA], AF.Silu)
            nc.scalar.activation(ft[0][0:64, CA:PADFLAT], xpad[0:64, CA:PADFLAT], AF.Silu)
            # the shifted silu half is a 1-col-shifted COPY of the plain
            # silu: SB->SB DMA halves on the otherwise-idle device replace
            # two ACT passes (the ACT queue is the startup bottleneck)
            nc.sync.dma_start(ft[0][64:128, 0:CA - 1], ft[0][0:64, 1:CA])
            nc.sync.dma_start(xpad[64:128, CA:PADFLAT], xpad[0:C, CA:PADFLAT])
            nc.sync.dma_start(ft[0][64:128, CA - 1:PADFLAT - 1], ft[0][0:64, CA:PADFLAT])
            nc.sync.dma_start(w_sb[0][:, 3 * OC:6 * OC],
                              wt_in[:, 3:6, :].rearrange("p j o -> p (j o)").bitcast(tdt[0]))
            for t in range(1, NT):
                nc.sync.dma_start(
                    w_sb[t][:],
                    wt_in[:, _TILE_J0[t]:_TILE_J1[t], :].rearrange("p j o -> p (j o)").bitcast(tdt[t]))
            # clamp precursors: v0 = relu(u) (right features), w11 =
            # relu(11-u) (left features); features are relu(bias - prec)^3
            v0 = xfpool.tile([128, PADFLAT], F32, tag="v0")
            w11 = xfpool.tile([128, PADFLAT], F32, tag="w11")
            # critical chain to the first cube tile: v0A -> relu1A -> sq/mul
            # (v0B only feeds relu1B, which has slack; keep it off the path)
            nc.scalar.activation(v0[:, 0:CA], xpad[:, 0:CA], AF.Relu,
                                 scale=1.0 / HSTEP, bias=bias_sb[:, NT - 1:NT])

            def src_for(fidx):
                return v0 if _FEATS[fidx][0] == "R" else w11

            def cube_tile(i, splitA=False, sq_dve=False):
                lo, hi = _PAIRS[i - 1]
                s_lo, s_hi = src_for(lo), src_for(hi)
                cols = [(0, CA), (CA, PADFLAT)] if splitA else [(0, PADFLAT)]
                for (c0, c1) in cols:
                    if s_lo is s_hi:
                        nc.scalar.activation(ft[i][:, c0:c1], s_lo[:, c0:c1], AF.Relu,
                                             scale=-1.0, bias=bias_sb[:, i - 1:i])
                    else:
                        nc.scalar.activation(ft[i][0:64, c0:c1], s_lo[0:64, c0:c1],
                                             AF.Relu, scale=-1.0, bias=bias_sb[0:64, i - 1:i])
                        nc.scalar.activation(ft[i][64:128, c0:c1], s_hi[64:128, c0:c1],
                                             AF.Relu, scale=-1.0, bias=bias_sb[64:128, i - 1:i])
                tmp = tmppool.tile([128, PADFLAT], tdt[i], tag="cube_tmp")
                if sq_dve:
                    nc.vector.tensor_mul(tmp[:], ft[i][:], ft[i][:])
                    nc.vector.tensor_mul(ft[i][:], tmp[:], ft[i][:])
                elif splitA:
                    nc.vector.tensor_mul(tmp[:, 0:CA], ft[i][:, 0:CA], ft[i][:, 0:CA])
                    nc.vector.tensor_mul(ft[i][:, 0:CA], tmp[:, 0:CA], ft[i][:, 0:CA])
                    nc.vector.tensor_mul(tmp[:, CA:PADFLAT], ft[i][:, CA:PADFLAT],
                                         ft[i][:, CA:PADFLAT])
                    nc.vector.tensor_mul(ft[i][:, CA:PADFLAT], tmp[:, CA:PADFLAT],
                                         ft[i][:, CA:PADFLAT])
                else:
                    nc.scalar.activation(tmp[:], ft[i][:], AF.Square)
                    nc.vector.tensor_mul(ft[i][:], tmp[:], ft[i][:])

            tmp1 = tmppool.tile([128, PADFLAT], tdt[1], tag="cube_tmp")
            nc.scalar.activation(ft[1][:, 0:CA], v0[:, 0:CA], AF.Relu,
                                 scale=-1.0, bias=bias_sb[:, 0:1])
            nc.vector.tensor_mul(tmp1[:, 0:CA], ft[1][:, 0:CA], ft[1][:, 0:CA])
            nc.vector.tensor_mul(ft[1][:, 0:CA], tmp1[:, 0:CA], ft[1][:, 0:CA])
            nc.scalar.activation(v0[:, CA:PADFLAT], xpad[:, CA:PADFLAT], AF.Relu,
                                 scale=1.0 / HSTEP, bias=bias_sb[:, NT - 1:NT])
            nc.scalar.activation(ft[1][:, CA:PADFLAT], v0[:, CA:PADFLAT], AF.Relu,
                                 scale=-1.0, bias=bias_sb[:, 0:1])
            nc.vector.tensor_mul(tmp1[:, CA:PADFLAT], ft[1][:, CA:PADFLAT],
                                 ft[1][:, CA:PADFLAT])
            nc.vector.tensor_mul(ft[1][:, CA:PADFLAT], tmp1[:, CA:PADFLAT],
                                 ft[1][:, CA:PADFLAT])
            nc.scalar.activation(w11[:, 0:CA], xpad[:, 0:CA], AF.Relu,
                                 scale=-1.0 / HSTEP, bias=bias_sb[:, NT - 1:NT])
            nc.scalar.activation(w11[:, CA:PADFLAT], xpad[:, CA:PADFLAT], AF.Relu,
                                 scale=-1.0 / HSTEP, bias=bias_sb[:, NT - 1:NT])
            cube_tile(2, sq_dve=True)
            for i in range(3, NT):
                cube_tile(i)

            psum = [psumpool.tile([128, NCHUNK], F32, tag=f"pb{k}", name=f"pb{k}")
                    for k in range(NCH)]

            def rhs_ap(t, rows, off, k):
                base = off + k * NROW * WP
                return (ft[t][0:rows, base:base + NROW * WP]
                        .rearrange("p (r c) -> p r c", c=WP)[:, :, 0:W])

            def emit_group(jobs, j0, first, last, banks=range(NCH)):
                """Bank-major matmuls for one tile-group; drains after the
                final group's banks so the store pipeline staggers."""
                for k in banks:
                    for jj, (t, rows, off, blocks) in enumerate(jobs):
                        lhsT = w_sb[t][0:rows, (j0 + jj - _TILE_J0[t]) * OC:
                                       (j0 + jj - _TILE_J0[t] + 1) * OC]
                        nc.tensor.matmul(psum[k][:], lhsT,
                                         rhs_ap(t, rows, off, k),
                                         start=(first and jj == 0),
                                         stop=(last and jj == len(jobs) - 1))
                    if last:
                        ob = opool.tile([128, NCHUNK], F32, tag=f"ob{k}", name=f"ob{k}")
                        if k % 2 == 1:
                            nc.vector.tensor_copy(ob[:], psum[k][:])
                        else:
                            nc.scalar.activation(ob[:], psum[k][:], AF.Copy)
                        nc.sync.dma_start(
                            out[:, NROW * k:NROW * k + NROW, :],
                            ob[:].rearrange("p (r c) -> p r c", c=W))

            # t0 half jobs | t0 full jobs (warm pads bridge the feature
            # production gaps without letting PE go idle) | t1..t6
            emit_group(_JOBS[0:3], 0, True, False, banks=range(0, 3))
            warms(NWARM2)
            emit_group(_JOBS[0:3], 0, True, False, banks=range(3, NCH))
            warms(NWARM3)
            emit_group(_JOBS[3:6], 3, False, False)
            warms(NWARM4)
            for t in range(1, NT):
                emit_group(_JOBS[_TILE_J0[t]:_TILE_J1[t]], _TILE_J0[t],
                           False, t == NT - 1)
    _split_excess_waits(nc)
    return nc


_CACHE = {}


def kernel(x, base_weight, spline_weight, spline_scaler):
    from concourse.bass_utils import run_bass_kernel_spmd

    x = np.ascontiguousarray(x, dtype=np.float32)
    wt, bias = _host_weights(
        np.asarray(base_weight, np.float32),
        np.asarray(spline_weight, np.float32),
        np.asarray(spline_scaler, np.float32),
    )
    if "nc" not in _CACHE:
        _CACHE["nc"] = _build_nc()
    nc = _CACHE["nc"]
    in_maps = [{"x": x[b], "wt": wt, "bias": bias} for b in range(B)]
    res = run_bass_kernel_spmd(nc, in_maps, list(range(B)))
    out = np.stack([res.results[b]["out"] for b in range(B)], axis=0)
    return out


# revision 55
# speedup vs baseline: 1.0025x; 1.0025x over previous
"""KANConv2d Trainium2 kernel (8-core data-parallel over batch).

Math: the efficient-kan B-spline bases on the uniform grid (knots at
u = 0..11, u = x/0.4 + 5.5) are combinations of truncated cubic powers.
The naive one-sided stack relu(u-m)^3 cancels ~2200-magnitude terms
down to O(1), which float32r's ~11-bit mantissa cannot survive (10%
error). Instead bases g=0..3 use right-side CLAMPED powers
relu(m - max(u,0))^3 and bases g=4..7 left-side relu(min(u,11) - m)^3:
the clamps make both telescopes exactly zero outside support and cap
term magnitudes at 343, so fp32r matmuls (4x the fp32 PE rate) keep
the end-to-end error at ~1.5e-2. The layer becomes a 3x3 conv over
15*64 feature channels [silu(x); 14 clamped cubes] with host-folded
weights, run as shifted fp32r matmuls accumulating in PSUM over a
58-wide zero-padded flattened feature map.

Feature tiles (128 partitions = 2 channel groups of 64):
  t0    = [silu(map); silu(map shifted left 1 col)]
  t1..7 = clamped-cube pairs (relu via per-partition bias against the
          shared precursors v0 = relu(u), w11 = relu(11-u))
The shifted silu copy lets one 128-row matmul cover taps kw=0 and kw=1
simultaneously, so t0 needs 6 matmul jobs per 9 taps instead of 9+
half-empty ones.  Matmul rhs uses a strided [8 rows x 56 cols] view so
no garbage pad columns are streamed (448 rows/job/bank). PE warm-up
matmuls keep the busy-streak alive so real matmuls are cost-priced at
full p-state (dispatch-time pricing against the 36-deep PE queue).
"""
import math
import numpy as np

import concourse.bass as bass
import concourse.mybir as mybir
import concourse.tile as tile
from concourse.tile import TileContext

# ---- problem constants (hardcoded per harness contract) ----
B, C, H, W = 8, 64, 56, 56
OC = 128
GRID_SIZE, SPLINE_ORDER = 5, 3
GRID_LO, GRID_HI = -1.0, 1.0
HSTEP = (GRID_HI - GRID_LO) / GRID_SIZE        # 0.4
T0 = GRID_LO - SPLINE_ORDER * HSTEP            # -2.2
NM = GRID_SIZE + 2 * SPLINE_ORDER + 1          # 12 truncated powers
WP = W + 2                                     # 58 padded width
PADFLAT = WP * WP + 4                          # 3368 padded/rounded
NROW = 8                                       # out rows per psum bank
NCHUNK = NROW * W                              # 448 (<=512 psum fp32)
NCH = 7                                        # chunks: 7*8 = 56 rows
NT = 8                                         # feature tiles
F32 = mybir.dt.float32
F32R = mybir.dt.float32r
M_DT = mybir.dt.float32r   # matmul operand dtype (float32r or float32)
F32R_TILES = set(range(8))  # feature tiles using f32r (debug bisect)


NWARM_TINY = 30                                # burn the 36-deep price burst
NWARM = 30                                     # PE warm-up matmuls (medium)
NWARM2 = 0                                     # pad before t0half banks 3-6
NWARM3 = 7                                     # pad before t0full
NWARM4 = 28                                    # pad before t1 (cube pipeline fill)
CA = 1682                                      # column split (banks 0-2 | 3-6)

# Two-sided clamped truncated-power features (f32r-safe): bases g=0..3 via
# right-side powers relu(m - max(u,0))^3 (m=1..7), bases g=4..7 via
# left-side powers relu(min(u,11) - m)^3 (m=4..10). Both telescopes are
# exactly zero outside support thanks to the clamps, and term magnitudes
# stay <= 343 so float32r's ~11-bit mantissa noise stays ~1e-2 under the
# massive-cancellation blowup of the naive one-sided stack.
# feature list: ("R", m) -> relu(m - v0)^3 with v0 = relu(u)
#               ("L", m) -> relu((11-m) - w11)^3 with w11 = relu(11-u)
_FEATS = [("R", m) for m in range(1, 8)] + [("L", m) for m in range(4, 11)]
# cube tile pairing: (lo feature idx, hi feature idx) per tile 1..7
_PAIRS = [(0, 1), (2, 3), (4, 5), (6, 7), (8, 9), (10, 11), (12, 13)]


def _jobs():
    """Matmul job list: (tile, rows, off, blocks). blocks[i] covers lhsT
    rows 64i:64i+64; ("base", kh, kw) or ("feat", fidx, kh, kw)."""
    jobs = []
    # t0 half jobs first (need only the unshifted silu half)
    for kh in range(3):
        jobs.append((0, 64, kh * WP + 2, [("base", kh, 2)]))
    # t0 full jobs: rows 0:64 tap kw=0, rows 64:128 (shifted copy) tap kw=1
    for kh in range(3):
        jobs.append((0, 128, kh * WP, [("base", kh, 0), ("base", kh, 1)]))
    for i in range(1, NT):
        lo, hi = _PAIRS[i - 1]
        for s in range(9):
            kh, kw = s // 3, s % 3
            jobs.append((i, 128, kh * WP + kw,
                         [("feat", lo, kh, kw), ("feat", hi, kh, kw)]))
    return jobs


_JOBS = _jobs()
NJ = len(_JOBS)                                # 60
# jobs per feature tile (for weight DMA splitting)
_TILE_J0 = [min(j for j, job in enumerate(_JOBS) if job[0] == t) for t in range(NT)]
_TILE_J1 = [max(j for j, job in enumerate(_JOBS) if job[0] == t) + 1 for t in range(NT)]


def _patch_tile_drain():
    """walrus in this container rejects sem waits on InstDrain (CTRL_NO
    struct): move the end-of-kernel drain waits onto single-wait NOPs."""
    import bass_rust

    def _drain_and_barrier(self, tick_clock, wait_clock):
        collector = self.nc.sync.nop(nofuse=True, hint="drain_waits")
        wait_clock.add_sem_waits(
            collector.ins, bass_rust.ScopedClock({None: tick_clock.global_clock})
        )
        waits = list(collector.ins.sync_info.on_wait)
        collector.ins.sync_info = mybir.SyncInfo(on_wait=waits[:1], on_update=[])
        for w in waits[1:]:
            n = self.nc.sync.nop(nofuse=True, hint="drain_waits")
            n.ins.sync_info = mybir.SyncInfo(on_wait=[w], on_update=[])
        self.nc.sync.drain()
        self.nc.all_engine_barrier()
        popped = self.nc._tile_sem_poison_stack.pop()
        assert popped is self._sem_poison

    TileContext._drain_and_barrier = _drain_and_barrier


_patch_tile_drain()


def _split_excess_waits(nc):
    """This walrus caps sync waits at 1/instruction (2 for EventSemaphore).
    Spill excess waits onto EventSemaphore insts inserted just before the
    overloaded instruction on the same engine."""
    import bass_rust

    counter = [0]
    for func in nc.m.functions:
        for bb in func.blocks:
            insts = bb.instructions
            out = []
            changed = False
            for inst in insts:
                si = getattr(inst, "sync_info", None)
                waits = list(si.on_wait) if si is not None else []
                cap = 2 if isinstance(inst, bass_rust.InstEventSemaphore) else 1
                if len(waits) > cap:
                    excess = waits[cap:]
                    for i in range(0, len(excess), 2):
                        counter[0] += 1
                        ev = bass_rust.InstEventSemaphore(
                            name=f"evspill-{counter[0]}",
                            engine=inst.engine,
                            ins=[], outs=[],
                            sync_info=mybir.SyncInfo(
                                on_wait=excess[i:i + 2], on_update=[]),
                        )
                        out.append(ev)
                    inst.sync_info = mybir.SyncInfo(
                        on_wait=waits[:cap], on_update=list(si.on_update))
                    changed = True
                out.append(inst)
            if changed:
                bb.instructions = out


def _feat_bias(fidx):
    """relu argument: bias - precursor (precursor = v0 for R, w11 for L)."""
    side, m = _FEATS[fidx]
    return float(m) if side == "R" else float(11 - m)


def _host_weights(base_weight, spline_weight, spline_scaler):
    """Fold spline bases into per-job lhsT blocks Wt[row, j, o]."""
    NB = GRID_SIZE + SPLINE_ORDER
    Tnew = np.zeros((NB, len(_FEATS)), dtype=np.float64)
    fmap = {f: i for i, f in enumerate(_FEATS)}
    for g in range(NB):
        for r in range(SPLINE_ORDER + 2):
            cr = ((-1) ** r) * math.comb(SPLINE_ORDER + 1, r) / 6.0
            key = ("R", g + 4 - r) if g <= 3 else ("L", g + r)
            if key in fmap:
                Tnew[g, fmap[key]] += cr
            # dropped features (R0, L11) are identically zero
    scaled = spline_weight.astype(np.float64) * spline_scaler.astype(np.float64)[..., None]
    W2 = np.einsum("oig,gf->oif", scaled, Tnew)                 # (O, 576, 14)
    W24 = W2.reshape(OC, C, 3, 3, len(_FEATS))
    bw4 = base_weight.astype(np.float64).reshape(OC, C, 3, 3)
    wt = np.zeros((128, NJ, OC), dtype=np.float32)
    for j, (t, rows, off, blocks) in enumerate(_JOBS):
        for bi, blk in enumerate(blocks):
            if blk[0] == "base":
                _, kh, kw = blk
                blockw = bw4[:, :, kh, kw]                      # (O, C)
            else:
                _, fidx, kh, kw = blk
                blockw = W24[:, :, kh, kw, fidx]                # (O, C)
            wt[64 * bi:64 * bi + 64, j, :] = blockw.T.astype(np.float32)
    bias = np.zeros((128, NT - 1), dtype=np.float32)
    for i in range(1, NT):
        lo, hi = _PAIRS[i - 1]
        bias[0:64, i - 1] = _feat_bias(lo)
        bias[64:128, i - 1] = _feat_bias(hi)
    return wt, bias


def _build_nc():
    nc = bass.Bass()
    tdt = [M_DT if t in F32R_TILES else F32 for t in range(NT)]
    x_in = nc.declare_dram_parameter("x", [C, H, W], F32, isOutput=False)
    wt_in = nc.declare_dram_parameter("wt", [128, NJ, OC], F32, isOutput=False)
    out = nc.declare_dram_parameter("out", [OC, H, W], F32, isOutput=True)

    AF = mybir.ActivationFunctionType
    with TileContext(nc) as tc:
        with (
            tc.tile_pool(name="w", bufs=1) as wpool,
            tc.tile_pool(name="xf", bufs=1) as xfpool,
            tc.tile_pool(name="ftc", bufs=4) as ftcpool,
            tc.tile_pool(name="tmp", bufs=2) as tmppool,
            tc.tile_pool(name="ob", bufs=1) as opool,
            tc.tile_pool(name="psum", bufs=1, space="PSUM") as psumpool,
        ):
            # relu bias vector built by on-chip memsets (no DMA slot)
            bias_sb = wpool.tile([128, NT], F32, tag="bias_sb")
            for i in range(1, NT):
                lo, hi = _PAIRS[i - 1]
                nc.gpsimd.memset(bias_sb[0:64, i - 1:i], _feat_bias(lo))
                nc.gpsimd.memset(bias_sb[64:128, i - 1:i], _feat_bias(hi))
            nc.gpsimd.memset(bias_sb[:, NT - 1:NT], -T0 / HSTEP)  # precursor bias 5.5
            xpad = xfpool.tile([128, PADFLAT], F32, tag="xpad")
            xv = xpad[:, :WP * WP].rearrange("p (r c) -> p r c", c=WP)
            # zero only the pad border of rows 0:64 (the interior is about
            # to be DMA'd over; a full memset would delay that DMA). Rows
            # 64:128 get everything via the on-chip duplication DMA.
            nc.gpsimd.memset(xpad[0:C, 0:WP - 1], 0.0)
            # right border of row r + left border of row r+1 are adjacent
            nc.gpsimd.memset(
                xpad[0:C, WP - 1:WP - 1 + 57 * WP]
                .rearrange("p (r c) -> p r c", c=WP)[:, :, 0:2], 0.0)
            nc.gpsimd.memset(xpad[0:C, 57 * WP + 1:PADFLAT], 0.0)

            # PE warm-up: instruction costs are priced at engine dispatch
            # against the PE busy-ramp (pe_busy_start = start of the current
            # busy streak), and the 36-deep PE queues are priced in a burst
            # at streak start. Any PE idle gap therefore poisons the next
            # ~36 matmuls with the slow-p-state price. The warm block keeps
            # PE busy from ~2us until the first real matmul is ready.
            warm = wpool.tile([64, 128], F32, tag="warm")
            nc.gpsimd.memset(warm[:], 0.0)
            psum_w = psumpool.tile([64, 64], F32, tag="pwarm")

            def warms(n, ap=64):
                for _ in range(n):
                    nc.tensor.matmul(psum_w[0:64, 0:ap], warm[0:64, 0:64].bitcast(M_DT),
                                     warm[0:64, 64:64 + ap].bitcast(M_DT),
                                     start=True, stop=True)

            warms(NWARM_TINY, ap=8)
            warms(NWARM)

            w_sb = [wpool.tile([128, (_TILE_J1[t] - _TILE_J0[t]) * OC], tdt[t],
                               tag=f"w{t}", name=f"w{t}") for t in range(NT)]

            # Single SP queue; device transfer order == this order (the dup
            # DMAs are "ready" by the time the static scheduler reaches
            # them, so nothing gets hoisted past them). The t0-half-job
            # weights (64 rows, 3 jobs) ride a tiny early DMA so the first
            # real matmul gate is x + silu, not the full wt0 transfer.
            nc.sync.dma_start(xv[0:C, 1:29, 1:W + 1], x_in[:, 0:28, :])
            nc.sync.dma_start(w_sb[0][0:64, 0:3 * OC],
                              wt_in[0:64, 0:3, :].rearrange("p j o -> p (j o)").bitcast(tdt[0]))
            nc.sync.dma_start(xpad[64:128, 0:CA], xpad[0:C, 0:CA])
            nc.sync.dma_start(xv[0:C, 29:H + 1, 1:W + 1], x_in[:, 28:H, :])

            ft = [xfpool.tile([128, PADFLAT], tdt[0], tag="ft0", name="ft0")]
            ft += [ftcpool.tile([128, PADFLAT], tdt[t], tag="ftc", name=f"ft{t}")
                   for t in range(1, NT)]
            # Early passes split at the bank-2/3 column boundary so matmuls
            # on banks 0-2 can start while the B half is still computing.
            nc.scalar.activation(ft[0][0:64, 0:CA], xpad[0:64, 0:CA], AF.Silu)
            nc.scalar.activation(ft[0][0:64, CA:PADFLAT], xpad[0:64, CA:PADFLAT], AF.Silu)
            # the shifted silu half is a 1-col-shifted COPY of the plain
            # silu: SB->SB DMA halves on the otherwise-idle device replace
            # two ACT passes (the ACT queue is the startup bottleneck)
            nc.sync.dma_start(ft[0][64:128, 0:CA - 1], ft[0][0:64, 1:CA])
            nc.sync.dma_start(xpad[64:128, CA:PADFLAT], xpad[0:C, CA:PADFLAT])
            nc.sync.dma_start(ft[0][64:128, CA - 1:PADFLAT - 1], ft[0][0:64, CA:PADFLAT])
            nc.sync.dma_start(w_sb[0][:, 3 * OC:6 * OC],
                              wt_in[:, 3:6, :].rearrange("p j o -> p (j o)").bitcast(tdt[0]))
            for t in range(1, NT):
                nc.sync.dma_start(
                    w_sb[t][:],
                    wt_in[:, _TILE_J0[t]:_TILE_J1[t], :].rearrange("p j o -> p (j o)").bitcast(tdt[t]))
            # clamp precursors: v0 = relu(u) (right features), w11 =
            # relu(11-u) (left features); features are relu(bias - prec)^3
            v0 = xfpool.tile([128, PADFLAT], F32, tag="v0")
            w11 = xfpool.tile([128, PADFLAT], F32, tag="w11")
            # critical chain to the first cube tile: v0A -> relu1A -> sq/mul
            # (v0B only feeds relu1B, which has slack; keep it off the path)
            nc.scalar.activation(v0[:, 0:CA], xpad[:, 0:CA], AF.Relu,
                                 scale=1.0 / HSTEP, bias=bias_sb[:, NT - 1:NT])

            def src_for(fidx):
                return v0 if _FEATS[fidx][0] == "R" else w11

            def cube_tile(i, splitA=False, sq_dve=False):
                lo, hi = _PAIRS[i - 1]
                s_lo, s_hi = src_for(lo), src_for(hi)
                cols = [(0, CA), (CA, PADFLAT)] if splitA else [(0, PADFLAT)]
                for (c0, c1) in cols:
                    if s_lo is s_hi:
                        nc.scalar.activation(ft[i][:, c0:c1], s_lo[:, c0:c1], AF.Relu,
                                             scale=-1.0, bias=bias_sb[:, i - 1:i])
                    else:
                        nc.scalar.activation(ft[i][0:64, c0:c1], s_lo[0:64, c0:c1],
                                             AF.Relu, scale=-1.0, bias=bias_sb[0:64, i - 1:i])
                        nc.scalar.activation(ft[i][64:128, c0:c1], s_hi[64:128, c0:c1],
                                             AF.Relu, scale=-1.0, bias=bias_sb[64:128, i - 1:i])
                tmp = tmppool.tile([128, PADFLAT], tdt[i], tag="cube_tmp")
                if sq_dve:
                    nc.vector.tensor_mul(tmp[:], ft[i][:], ft[i][:])
                    nc.vector.tensor_mul(ft[i][:], tmp[:], ft[i][:])
                elif splitA:
                    nc.vector.tensor_mul(tmp[:, 0:CA], ft[i][:, 0:CA], ft[i][:, 0:CA])
                    nc.vector.tensor_mul(ft[i][:, 0:CA], tmp[:, 0:CA], ft[i][:, 0:CA])
                    nc.vector.tensor_mul(tmp[:, CA:PADFLAT], ft[i][:, CA:PADFLAT],
                                         ft[i][:, CA:PADFLAT])
                    nc.vector.tensor_mul(ft[i][:, CA:PADFLAT], tmp[:, CA:PADFLAT],
                                         ft[i][:, CA:PADFLAT])
                else:
                    nc.scalar.activation(tmp[:], ft[i][:], AF.Square)
                    nc.vector.tensor_mul(ft[i][:], tmp[:], ft[i][:])

            tmp1 = tmppool.tile([128, PADFLAT], tdt[1], tag="cube_tmp")
            nc.scalar.activation(ft[1][:, 0:CA], v0[:, 0:CA], AF.Relu,
                                 scale=-1.0, bias=bias_sb[:, 0:1])
            nc.vector.tensor_mul(tmp1[:, 0:CA], ft[1][:, 0:CA], ft[1][:, 0:CA])
            nc.vector.tensor_mul(ft[1][:, 0:CA], tmp1[:, 0:CA], ft[1][:, 0:CA])
            nc.scalar.activation(v0[:, CA:PADFLAT], xpad[:, CA:PADFLAT], AF.Relu,
                                 scale=1.0 / HSTEP, bias=bias_sb[:, NT - 1:NT])
            nc.scalar.activation(ft[1][:, CA:PADFLAT], v0[:, CA:PADFLAT], AF.Relu,
                                 scale=-1.0, bias=bias_sb[:, 0:1])
            nc.vector.tensor_mul(tmp1[:, CA:PADFLAT], ft[1][:, CA:PADFLAT],
                                 ft[1][:, CA:PADFLAT])
            nc.vector.tensor_mul(ft[1][:, CA:PADFLAT], tmp1[:, CA:PADFLAT],
                                 ft[1][:, CA:PADFLAT])
            nc.scalar.activation(w11[:, 0:CA], xpad[:, 0:CA], AF.Relu,
                                 scale=-1.0 / HSTEP, bias=bias_sb[:, NT - 1:NT])
            nc.scalar.activation(w11[:, CA:PADFLAT], xpad[:, CA:PADFLAT], AF.Relu,
                                 scale=-1.0 / HSTEP, bias=bias_sb[:, NT - 1:NT])
            cube_tile(2, sq_dve=True)
            for i in range(3, NT):
                cube_tile(i)

            psum = [psumpool.tile([128, NCHUNK], F32, tag=f"pb{k}", name=f"pb{k}")
                    for k in range(NCH)]

            def rhs_ap(t, rows, off, k):
                base = off + k * NROW * WP
                return (ft[t][0:rows, base:base + NROW * WP]
                        .rearrange("p (r c) -> p r c", c=WP)[:, :, 0:W])

            def emit_group(jobs, j0, first, last, banks=range(NCH)):
                """Bank-major matmuls for one tile-group; drains after the
                final group's banks so the store pipeline staggers."""
                for k in banks:
                    for jj, (t, rows, off, blocks) in enumerate(jobs):
                        lhsT = w_sb[t][0:rows, (j0 + jj - _TILE_J0[t]) * OC:
                                       (j0 + jj - _TILE_J0[t] + 1) * OC]
                        nc.tensor.matmul(psum[k][:], lhsT,
                                         rhs_ap(t, rows, off, k),
                                         start=(first and jj == 0),
                                         stop=(last and jj == len(jobs) - 1))
                    if last:
                        ob = opool.tile([128, NCHUNK], F32, tag=f"ob{k}", name=f"ob{k}")
                        if k % 2 == 1:
                            nc.vector.tensor_copy(ob[:], psum[k][:])
                        else:
                            nc.scalar.activation(ob[:], psum[k][:], AF.Copy)
                        nc.sync.dma_start(
                            out[:, NROW * k:NROW * k + NROW, :],
                            ob[:].rearrange("p (r c) -> p r c", c=W))

            # t0 half jobs | t0 full jobs (warm pads bridge the feature
            # production gaps without letting PE go idle) | t1..t6
            emit_group(_JOBS[0:3], 0, True, False, banks=range(0, 3))
            warms(NWARM2)
            emit_group(_JOBS[0:3], 0, True, False, banks=range(3, NCH))
            warms(NWARM3)
            emit_group(_JOBS[3:6], 3, False, False)
            warms(NWARM4)
            for t in range(1, NT):
                emit_group(_JOBS[_TILE_J0[t]:_TILE_J1[t]], _TILE_J0[t],
                           False, t == NT - 1)
    _split_excess_waits(nc)
    return nc


_CACHE = {}


def kernel(x, base_weight, spline_weight, spline_scaler):
    from concourse.bass_utils import run_bass_kernel_spmd

    x = np.ascontiguousarray(x, dtype=np.float32)
    wt, bias = _host_weights(
        np.asarray(base_weight, np.float32),
        np.asarray(spline_weight, np.float32),
        np.asarray(spline_scaler, np.float32),
    )
    if "nc" not in _CACHE:
        _CACHE["nc"] = _build_nc()
    nc = _CACHE["nc"]
    in_maps = [{"x": x[b], "wt": wt, "bias": bias} for b in range(B)]
    res = run_bass_kernel_spmd(nc, in_maps, list(range(B)))
    out = np.stack([res.results[b]["out"] for b in range(B)], axis=0)
    return out
